# revision 18
# baseline (speedup 1.0000x reference)
"""Trainium2 Bass kernel for nn_EnhancedFractionalPINO.

Math rewrite (host-side, exact):
  * GL fractional conv is linear -> folded into Ws1:
      W1'[t,m] = sum_j w_j Ws1[t+j, m]  (causal correlation), plus a
      512x512 tail matrix for the cross-batch halo contribution.
  * spectral L2 and neural L1 have no nonlinearity between them:
      W23 = Ws2 @ Wn1 (512x512), b23 = bs2 @ Wn1 + bn1.
  * ifft2 is linear -> folded into Wn3:  G[f,:] = Re(ifft2(Wn3[f] img)).

Kernel per core (batch-parallel, 32 batches/core):
  fft2 of 96+halo images via DFT matmuls -> fbuf (flat f signal, f16)
  h0 = f @ W1' + tail @ Wtail   (W1' fp8-e3m4, per-col scales folded into
                                 the relu activation's per-partition scale)
  h1 = relu(h0 @ W23 + b23); h2 = relu(h1 @ W4 + b4)
  out = h2 @ G + gb             (G fp8-e3m4, per-row scales folded into
                                 L4's activation scale; ifft2 pre-applied)
  Everything feature-major: weights stationary (lhsT), batch=32 moving.
  Output leaves in [128, 96chunk, 32batch] layout; host transposes.
"""

import numpy as np
import ml_dtypes

import concourse.bass as bass
import concourse.mybir as mybir
import concourse.tile as tile
from concourse import bacc
from concourse.bass_utils import run_bass_kernel_spmd

F32 = mybir.dt.float32
F16 = mybir.dt.float16
F8E3 = mybir.dt.float8e3
AF = mybir.ActivationFunctionType

B, C, H, W = 256, 3, 64, 64
MODES = C * H * W              # 12288
ALPHA = 0.5
NTOT = B * MODES
NCORE = 8
BS = B // NCORE                # 32 batches per core
NIMG = BS * C                  # 96 images per core
NSLOT = NIMG + 2               # halo + 96 images + zero pad
KTAPS = 512                    # truncated GL taps
NCH = 96                       # 128-elem chunks per batch

GAM1, GAM2, GAM3 = 1.0 / 8, 0.5, 4.0
E3MAX = 14.88                  # 0.96 * e3m4 max (15.5)

W1_FP8 = True
G_FP8 = True


# ---------------------------------------------------------------- host consts
def _dft_consts():
    jk = np.outer(np.arange(64), np.arange(64)).astype(np.float64)
    Cm = np.cos(2 * np.pi * jk / 64)
    Sm = np.sin(2 * np.pi * jk / 64)
    # [cswi | cmf | msf | ones(row0)] packed into one DMA
    pack = np.zeros((64, 256 + BS))
    pack[:, 0:64] = Cm
    pack[:, 64:128] = Sm
    pack[:, 128:192] = Cm
    pack[:, 192:256] = -Sm
    pack[0, 256:256 + BS] = 1.0
    return {"fftpk": np.ascontiguousarray(pack, dtype=np.float16)}


def _gl_w():
    j = np.arange(1, KTAPS, dtype=np.float64)
    return np.concatenate([[1.0], np.cumprod((j - 1.0 - ALPHA) / j)])


def _col_major(Wm, ktiles):
    """[K, M] -> [128, ktiles, M] with partition = K % 128."""
    K, M = Wm.shape
    assert K == ktiles * 128
    return np.ascontiguousarray(Wm.reshape(ktiles, 128, M).transpose(1, 0, 2))


def _pm(v):
    """[512] -> [128, 4] f32 (partition, m-tile)."""
    return np.ascontiguousarray(v.reshape(4, 128).T, dtype=np.float32)


def _prep_weights(Ws1, bs1, Ws2, bs2, Wn1, bn1, Wn2, bn2, Wn3, bn3):
    w = _gl_w()
    hscale = (1.0 / (NTOT - 1)) ** (-ALPHA)
    W1 = Ws1.astype(np.float64) * hscale

    L = 1 << 15
    wf = np.fft.rfft(w, L).conj()[:, None]
    W1p = np.fft.irfft(np.fft.rfft(W1, L, axis=0) * wf, L, axis=0)[:MODES]
    Wtail = np.zeros((KTAPS, 512))
    for p in range(1, KTAPS):
        Wtail[p] = w[KTAPS - p:] @ W1[:p]

    W23 = Ws2.astype(np.float64) @ Wn1.astype(np.float64)
    b23 = bs2.astype(np.float64) @ Wn1.astype(np.float64) + bn1
    G = np.real(np.fft.ifft2(Wn3.astype(np.float64).reshape(512, 3, 64, 64),
                             axes=(-2, -1))).reshape(512, MODES)
    gb = np.real(np.fft.ifft2(bn3.astype(np.float64).reshape(3, 64, 64),
                              axes=(-2, -1))).reshape(-1)

    if W1_FP8:
        s1 = np.abs(W1p).max(axis=0) / E3MAX
        w1q = (W1p / s1).astype(ml_dtypes.float8_e3m4)
        w1d = _col_major(w1q, 96).view(np.uint8)
    else:
        s1 = np.ones(512)
        w1d = _col_major(W1p, 96).astype(np.float16)

    if G_FP8:
        sq = np.abs(G).max(axis=1) / E3MAX
        gq = (G / sq[:, None]).astype(ml_dtypes.float8_e3m4)
        gd = np.ascontiguousarray(
            gq.reshape(4, 128, 96, 128).transpose(1, 0, 2, 3)).view(np.uint8)
    else:
        sq = np.full(512, 1.0 / 256)
        gd = np.ascontiguousarray(
            (G / sq[:, None]).reshape(4, 128, 96, 128).transpose(1, 0, 2, 3)
        ).astype(np.float16)

    f16 = lambda a: np.ascontiguousarray(a, dtype=np.float16)
    wpack = np.concatenate([_col_major(W23 / GAM1, 4),
                            _col_major(Wn2.astype(np.float64) / GAM2, 4)],
                           axis=1)                          # [128, 8, 512]
    spack = np.concatenate([_pm(s1 * GAM1), _pm(bs1 * GAM1),
                            _pm(b23 * GAM2), _pm(GAM3 * sq),
                            _pm(bn2 * GAM3 * sq)], axis=1)  # [128, 20]
    return {
        "w1q": w1d,
        "wtl8": _col_major(
            (Wtail / s1[None, :]).astype(ml_dtypes.float8_e3m4), 4
        ).view(np.uint8),
        "wpk": f16(wpack),
        "gq": gd,
        "spk": np.ascontiguousarray(spack, dtype=np.float32),
        "gb": f16((gb * GAM3).reshape(1, MODES)),
    }


# ---------------------------------------------------------------- bass module
_NC_CACHE = {}


def _build_nc():
    nc = bacc.Bacc("TRN2", target_bir_lowering=False, debug=False,
                   num_devices=NCORE)

    def din(name, shape, dt=F16):
        return nc.dram_tensor(name, shape, dt, kind="ExternalInput")

    d_x = din("ximgs", (64, NSLOT, 64))
    d_fpk = din("fftpk", (64, 256 + BS))
    d_w1 = din("w1q", (128, 96, 512), F8E3 if W1_FP8 else F16)
    d_wtl = din("wtl8", (128, 4, 512), F8E3)
    d_wpk = din("wpk", (128, 8, 512))
    d_g = din("gq", (128, 4, 96, 128), F8E3 if G_FP8 else F16)
    d_spk = din("spk", (128, 20), F32)
    d_gb = din("gb", (1, MODES))
    d_out = nc.dram_tensor("out", (128, NCH, BS), F16, kind="ExternalOutput")

    with tile.TileContext(nc) as tc:
        with tc.tile_pool(name="cpool", bufs=1) as cpool, \
             tc.tile_pool(name="bigpool", bufs=1) as bigpool:
            fpk = cpool.tile([64, 256 + BS], F16, tag="fpk")
            spk = cpool.tile([128, 20], F32, tag="spk")
            gbs = cpool.tile([1, MODES], F16, tag="gbs")
            cswi, cmf, msf = fpk[:, 0:128], fpk[:, 128:192], fpk[:, 192:256]
            ones1 = fpk[0:1, 256:256 + BS]
            sc1, b1s = spk[:, 0:4], spk[:, 4:8]
            b23s, sc4, b4s = spk[:, 8:12], spk[:, 12:16], spk[:, 16:20]
            # big DMAs in priority order on the sync queue; small packs on
            # the scalar queue so they don't hold up the stream
            xall = bigpool.tile([64, NSLOT, 64], F16, tag="xall")
            nc.sync.dma_start(xall[:], d_x[:])
            nc.scalar.dma_start(fpk[:], d_fpk[:])
            nc.scalar.dma_start(spk[:], d_spk[:])
            nc.scalar.dma_start(gbs[:], d_gb[:])

            w1s = bigpool.tile([128, 96, 512], F8E3 if W1_FP8 else F16,
                               tag="w1s")
            for ch in range(6):
                nc.sync.dma_start(w1s[:, 16 * ch:16 * (ch + 1), :],
                                  d_w1[:, 16 * ch:16 * (ch + 1), :])
            wtl = bigpool.tile([128, 4, 512], F8E3, tag="wtl")
            nc.sync.dma_start(wtl[:], d_wtl[:])
            wpk = bigpool.tile([128, 8, 512], F16, tag="wpk")
            nc.sync.dma_start(wpk[:], d_wpk[:])
            w23s = wpk[:, 0:4, :]
            w4s = wpk[:, 4:8, :]
            gs = bigpool.tile([128, 4, 96, 128], F8E3 if G_FP8 else F16,
                              tag="gs")
            gchunks = [(8 * i, 8 * (i + 1)) for i in range(10)] + \
                      [(80 + 4 * i, 84 + 4 * i) for i in range(4)]
            for c0, c1 in gchunks:
                nc.sync.dma_start(gs[:, :, c0:c1, :], d_g[:, :, c0:c1, :])

            fbuf = bigpool.tile([128, 3140], F16, tag="fbuf")
            a1 = bigpool.tile([128, 4, BS], F16, tag="a1")
            h1 = bigpool.tile([128, 4, BS], F16, tag="h1")
            h2 = bigpool.tile([128, 4, BS], F16, tag="h2")
            stage = bigpool.tile([128, NCH, BS], F16, tag="stage")

            # ========== phase F: fft2 -> fbuf ==============================
            with tc.tile_pool(name="gpool", bufs=6) as gpool, \
                 tc.tile_pool(name="ps1p", bufs=4, space="PSUM") as ps1p, \
                 tc.tile_pool(name="ps2p", bufs=3, space="PSUM") as ps2p:
                for grp in range(25):
                    n = 4 if grp < 24 else 2
                    psA = ps1p.tile([64, 512], F32, tag="psA")
                    for t in range(n):
                        i = grp * 4 + t
                        nc.tensor.matmul(psA[:, t * 128:(t + 1) * 128],
                                         xall[:, i, :], cswi,
                                         start=True, stop=True)
                    g1w = gpool.tile([64, 4, 128], F16, tag="g1w")
                    g1f = g1w[:, 0:n, :].rearrange("p a k -> p (a k)")
                    if grp % 2 == 0:
                        nc.scalar.copy(g1f, psA[:, 0:n * 128])
                    else:
                        nc.vector.tensor_copy(g1f, psA[:, 0:n * 128])
                    ps2 = ps2p.tile([64, 256], F32, tag="ps2")
                    nc.tensor.matmul(ps2[:, 0:n * 64], cmf,
                                     g1w[:, 0:n, 0:64], start=True, stop=False)
                    nc.tensor.matmul(ps2[:, 0:n * 64], msf,
                                     g1w[:, 0:n, 64:128], start=False,
                                     stop=True)
                    p2v = ps2.rearrange("p (k two) -> p k two", two=2)
                    if grp == 0:
                        nc.vector.tensor_copy(fbuf[0:64, 0:4], p2v[:, 28:32, 0])
                        nc.vector.tensor_copy(fbuf[64:128, 0:4],
                                              p2v[:, 28:32, 1])
                        nc.vector.tensor_copy(fbuf[0:64, 4:100],
                                              p2v[:, 32:128, 0])
                        nc.vector.tensor_copy(fbuf[64:128, 4:100],
                                              p2v[:, 32:128, 1])
                    else:
                        base = 4 + (grp * 4 - 1) * 32
                        cp = (nc.vector.tensor_copy if grp % 2 == 0
                              else nc.scalar.copy)
                        cp(fbuf[0:64, base:base + n * 32], p2v[:, 0:n * 32, 0])
                        cp(fbuf[64:128, base:base + n * 32],
                           p2v[:, 0:n * 32, 1])

            fview = fbuf[:, 4:4 + BS * 96].rearrange("p (b k) -> p b k", b=BS)
            ftail = fbuf[:, 0:BS * 96].rearrange("p (b k) -> p b k", b=BS)

            # ========== L1: h0 = f @ W1' + tail, relu ======================
            with tc.tile_pool(name="ps1m", bufs=1, space="PSUM") as ps1m:
                psL = [ps1m.tile([128, BS], F32, tag=f"psL{m}",
                                 name=f"psL{m}") for m in range(4)]
                for j in range(96):
                    for m in range(4):
                        nc.tensor.matmul(psL[m][:],
                                         w1s[:, j, m * 128:(m + 1) * 128],
                                         fview[:, :, j],
                                         start=(j == 0), stop=False)
                for jt in range(4):
                    for m in range(4):
                        nc.tensor.matmul(psL[m][:],
                                         wtl[:, jt, m * 128:(m + 1) * 128],
                                         ftail[:, :, jt],
                                         start=False, stop=(jt == 3))
                for m in range(4):
                    nc.scalar.activation(a1[:, m, :], psL[m][:], AF.Relu,
                                         bias=b1s[:, m:m + 1],
                                         scale=sc1[:, m:m + 1])

            # ========== L23 / L4: 512x512 layers ===========================
            with tc.tile_pool(name="ps2m", bufs=4, space="PSUM") as ps2m:
                for m in range(4):
                    acc = ps2m.tile([128, BS], F32, tag="acc23")
                    for k in range(4):
                        nc.tensor.matmul(acc[:],
                                         w23s[:, k, m * 128:(m + 1) * 128],
                                         a1[:, k, :],
                                         start=(k == 0), stop=(k == 3))
                    nc.scalar.activation(h1[:, m, :], acc[:], AF.Relu,
                                         bias=b23s[:, m:m + 1], scale=GAM2)
                for m in range(4):
                    acc = ps2m.tile([128, BS], F32, tag="acc4")
                    for k in range(4):
                        nc.tensor.matmul(acc[:],
                                         w4s[:, k, m * 128:(m + 1) * 128],
                                         h1[:, k, :],
                                         start=(k == 0), stop=(k == 3))
                    nc.scalar.activation(h2[:, m, :], acc[:], AF.Relu,
                                         bias=b4s[:, m:m + 1],
                                         scale=sc4[:, m:m + 1])

            # ========== L5: out = h2 @ G + gb (ifft2 pre-folded) ===========
            with tc.tile_pool(name="ps5m", bufs=8, space="PSUM") as ps5m:
                for c in range(NCH):
                    acc = ps5m.tile([128, BS], F32, tag="acc5")
                    for k in range(4):
                        nc.tensor.matmul(acc[:], gs[:, k, c, :], h2[:, k, :],
                                         start=(k == 0), stop=False)
                    nc.tensor.matmul(acc[:],
                                     gbs[0:1, c * 128:(c + 1) * 128],
                                     ones1, start=False, stop=True)
                    if c % 2 == 0:
                        nc.scalar.copy(stage[:, c, :], acc[:])
                    else:
                        nc.vector.tensor_copy(stage[:, c, :], acc[:])
                    lo = {63: 48, 79: 64, 87: 80, 95: 88}
                    if c in (15, 31, 47):
                        c0 = c - 15
                        nc.sync.dma_start(d_out[:, c0:c + 1, :],
                                          stage[:, c0:c + 1, :])
                    elif c in lo:
                        c0 = lo[c]
                        nc.sync.dma_start(d_out[:, c0:c + 1, :],
                                          stage[:, c0:c + 1, :])

    nc.compile()
    return nc


def _get_nc():
    key = (W1_FP8, G_FP8)
    if key not in _NC_CACHE:
        _NC_CACHE[key] = _build_nc()
    return _NC_CACHE[key]


def _make_in_maps(x, Ws1, bs1, Ws2, bs2, Wn1, bn1, Wn2, bn2, Wn3, bn3):
    shared = dict(_dft_consts())
    shared.update(_prep_weights(Ws1, bs1, Ws2, bs2, Wn1, bn1, Wn2, bn2,
                                Wn3, bn3))
    in_maps = []
    for g in range(NCORE):
        if g == 0:
            halo = np.zeros((1, 64, 64), np.float32)
        else:
            halo = x[g * BS - 1, 2][None]
        ximgs = np.concatenate(
            [halo, x[g * BS:(g + 1) * BS].reshape(NIMG, 64, 64),
             np.zeros((1, 64, 64), np.float32)]).astype(np.float16)
        in_maps.append({"ximgs": np.ascontiguousarray(
            ximgs.transpose(1, 0, 2)), **shared})
    return in_maps


def kernel(**inputs):
    ins = {k: np.asarray(v) for k, v in inputs.items()}
    x = np.ascontiguousarray(ins["x"], dtype=np.float32)
    nc = _get_nc()
    in_maps = _make_in_maps(
        x, ins["Ws1"], ins["bs1"], ins["Ws2"], ins["bs2"],
        ins["Wn1"], ins["bn1"], ins["Wn2"], ins["bn2"],
        ins["Wn3"], ins["bn3"])
    res = run_bass_kernel_spmd(nc, in_maps, list(range(NCORE)))
    out = np.empty((B, C, H, W), np.float32)
    for g in range(NCORE):
        st = np.asarray(res.results[g]["out"], dtype=np.float32) / GAM3
        out[g * BS:(g + 1) * BS] = st.transpose(2, 1, 0).reshape(
            BS, C, H, W)
    return out
